# revision 54
# baseline (speedup 1.0000x reference)
"""Trainium2 Bass kernel: paged int8-KV-cache GQA decode attention, 8-core SPMD.

Contract: kernel(**inputs) takes the FULL unsharded numpy inputs (as produced by
the reference setup_inputs) and returns the FULL [32, 4096] float32 output.

Strategy (data parallel over sequence-chunks, flash-decoding style):
  - The 32 sequences' token tiles (ceil(ctx/128) each) are carved into
    8 cores x SLOTS contiguous chunks; slot s has a fixed tile count L[s]
    shared by all cores (SPMD).  Every chunk computes unnormalized partials
    (PV^T, Z) and the host combines: out = sum(PV) / sum(Z).
  - KV lands in SBUF as RAW INT8 (the DMA engines charge max(src,dst) bytes,
    so int8->int8 halves HBM/DMA time vs the old inline int8->bf16 cast).
    Per (slot, 2-kvh-group) the host packs one contiguous DRAM block whose
    row p is [K(d=p) | V(t=p)], so each chunk is ONE big DMA.
  - On-chip the int8 is cast to bf16 for the matmuls, rate-matched across
    the two usable elementwise engines: DVE (~1.9 fe/ns, 2x_2p mode) takes
    K plus the tail ~25% of V, ACT (~1.1 fe/ns) the rest of V.  GpSimd is
    deliberately unused: it shares an SBUF port with the DVE and concurrent
    big copies collapse both engines ~3x (measured).  Casts are issued one
    chunk ahead of compute (in-order engine queues execute in dependency-
    ready order); chunk DMAs run 6 ahead on the sync HWDGE queue.
  - Per (slot, group of 2 kv heads):
      scores [128t, 2kvh, n, 4h] = per-tile matmuls(lhsT=K^T tile, rhs=q^T)
      s1 = scores * ksb  (DVE; ksb = k_scale*SCALE, zeroed beyond ctx)
      e  = exp(s1) in bf16 (ACT), ev = e * v_scale_vec (DVE)
      Z  = matmul(lhsT=e, rhs=ones) per kvh; pad tokens contribute exp(0)=1,
           corrected host-side via the known count
      PV = matmul(lhsT=V tile, rhs=ev) accumulated in PSUM as out^T [128d,4h]
  Softmax skips max-subtraction (scores are O(20) at most; fp32 exp is safe).
"""

import math
import os
from contextlib import ExitStack

# min-pop semaphore allocator: reuses a small set of semaphore handles,
# shrinking the end-of-kernel per-semaphore drain (safe here: no nested
# hardware loops, no collectives)
os.environ.setdefault("TRNINF_ENABLE_CUSTOMCOMMS_RDH_AG", "1")

import numpy as np

import sys
sys.path.insert(0, "/opt/trn_rl_repo")

import ml_dtypes  # noqa: E402

import concourse.bass as bass  # noqa: E402
import concourse.mybir as mybir  # noqa: E402
import concourse.tile as tile  # noqa: E402
from concourse import bacc  # noqa: E402
from concourse.bass_utils import run_bass_kernel_spmd  # noqa: E402

BF16 = ml_dtypes.bfloat16

B = 32
NUM_HEADS = 32
KVH = 8
D = 128
REP = NUM_HEADS // KVH  # 4
BLOCK_SIZE = 256
T = 4096
P = 128
SCALE = 1.0 / float(np.sqrt(D))
NCORES = 8

# per-chunk int8 bytes for a (2-kvh, n-tile) group: [K | V] rows
def _chunk_bytes(n):
    return 4 * n * P * P  # (2 kvh) * (K+V) * n tiles * 128 tok * 128 d


# ---------------------------------------------------------------------------
# host-side planning + packing
# ---------------------------------------------------------------------------

def _greedy_chunks(tiles, L):
    """Slot-by-slot, give the 8 largest remaining sequences a chunk of up to
    L[s] tiles.  Returns per-slot lists of (seq, start_tile, len) or None if
    some sequence is left uncovered."""
    rem = [int(t) for t in tiles]
    start = [0] * len(tiles)
    chunks = []
    for Ls in L:
        order = sorted(range(len(tiles)), key=lambda b: -rem[b])
        sc = []
        for c in range(NCORES):
            b = order[c]
            ln = min(rem[b], Ls)
            sc.append((b, start[b], ln))
            rem[b] -= ln
            start[b] += ln
        chunks.append(sc)
    if any(r > 0 for r in rem):
        return None
    return chunks


_PLAN_CACHE = {}


def _plan(context_lens):
    """Choose slot lengths L and the (core, slot) -> sequence-chunk map."""
    tiles = tuple(int(math.ceil(int(c) / P)) for c in context_lens)
    if tiles in _PLAN_CACHE:
        return _PLAN_CACHE[tiles]
    ts = sorted(tiles, reverse=True)
    # octile fallback (always feasible): whole sequences, 4 slots
    best = (ts[0] + ts[8] + ts[16] + ts[24], (ts[0], ts[8], ts[16], ts[24]))
    for L0 in range(max(2, ts[0] - 8), ts[0] + 1):
        for L1 in range(max(2, ts[8] - 8), min(L0, ts[8] + 4) + 1):
            for L2 in range(max(2, ts[16] - 7), min(L1, ts[16] + 4) + 1):
                for L3 in range(max(2, ts[24] - 5), min(L2, ts[24] + 4) + 1):
                    base = L0 + L1 + L2 + L3
                    for L4 in range(2, min(L3, 10) + 1):
                        for L5 in (0, *range(2, L4 + 1)):
                            l6r = (0,) if L5 == 0 else (0, *range(2, L5 + 1))
                            for L6 in l6r:
                                L = tuple(x for x in
                                          (L0, L1, L2, L3, L4, L5, L6) if x)
                                N = sum(L)
                                if N >= best[0]:
                                    continue
                                if _greedy_chunks(tiles, L) is not None:
                                    best = (N, L)
    L = list(best[1])
    chunks = _greedy_chunks(tiles, L)
    _PLAN_CACHE[tiles] = (L, chunks)
    return L, chunks


def _quantize(x):
    absmax = np.abs(x).max(axis=-1)
    scale = np.where(absmax > 0.0, absmax / 127.0, 1.0).astype(np.float32)
    xq = np.clip(np.round(x / scale[..., None]), -127.0, 127.0).astype(np.int32)
    return xq, scale


def _pack_inputs(inputs, L, chunks):
    q = inputs["q"].reshape(B, NUM_HEADS, D).astype(np.float32)
    k = inputs["k"].reshape(B, KVH, D).astype(np.float32)
    v = inputs["v"].reshape(B, KVH, D).astype(np.float32)
    kc = np.ascontiguousarray(inputs["k_cache_q"].reshape(-1, KVH, D))
    vc = np.ascontiguousarray(inputs["v_cache_q"].reshape(-1, KVH, D))
    ks = np.ascontiguousarray(inputs["k_scale"].reshape(-1, KVH)).astype(np.float32)
    vs = np.ascontiguousarray(inputs["v_scale"].reshape(-1, KVH)).astype(np.float32)
    bt = inputs["block_tables"]
    ctx = inputs["context_lens"]
    sm = inputs["slot_mapping"]

    # store_kvcache_int8: quantize the new token and scatter into the cache
    kq, ksn = _quantize(k)
    vq, vsn = _quantize(v)
    kc = kc.copy(); vc = vc.copy(); ks = ks.copy(); vs = vs.copy()
    kc[sm] = kq; vc[sm] = vq; ks[sm] = ksn; vs[sm] = vsn

    SLOTS = len(L)
    NTT = sum(L)
    offs = np.concatenate([[0], np.cumsum(L)])

    in_maps = []
    padcnt = np.zeros((NCORES, SLOTS), dtype=np.float64)
    for c in range(NCORES):
        kv_c = np.zeros((1, 16 * NTT * P * P), dtype=np.int8)
        # scales in global token-tile-major layout [P, KVH, NTT]
        ksb_c = np.zeros((P, KVH, NTT), dtype=np.float32)
        vsb_c = np.zeros((P, KVH, NTT), dtype=BF16)
        qt_c = np.zeros((P, SLOTS * 32), dtype=BF16)
        for s in range(SLOTS):
            b, t0, ln = chunks[s][c]
            n = L[s]
            nt = n * P
            o = int(offs[s])
            nvalid = max(0, min(int(ctx[b]) - t0 * P, ln * P))
            padcnt[c, s] = nt - nvalid
            if ln > 0:
                flat = (bt[b][:, None] * BLOCK_SIZE
                        + np.arange(BLOCK_SIZE, dtype=np.int64)[None, :]
                        ).reshape(-1)[t0 * P: t0 * P + ln * P]
                kg = np.zeros((nt, KVH, D), dtype=np.int8)
                vg = np.zeros((nt, KVH, D), dtype=np.int8)
                kg[: ln * P] = kc[flat]
                vg[: ln * P] = vc[flat]
                scg = np.zeros((nt, KVH), dtype=np.float32)
                svg = np.zeros((nt, KVH), dtype=np.float32)
                valid = (np.arange(nt) < nvalid)
                scg[: ln * P] = ks[flat] * SCALE
                svg[: ln * P] = vs[flat]
                scg *= valid[:, None]
                svg *= valid[:, None]
                kjdt = kg.transpose(1, 2, 0)                      # [KVH, D, nt]
                vpjid = vg.reshape(n, P, KVH, D).transpose(1, 2, 0, 3)
                for jh in range(KVH // 2):
                    co = 16 * o * P * P + _chunk_bytes(n) * jh
                    kb = kjdt[2 * jh: 2 * jh + 2].transpose(1, 0, 2
                                                            ).reshape(P, -1)
                    vb = vpjid[:, 2 * jh: 2 * jh + 2].reshape(P, -1)
                    kv_c[0, co: co + 4 * n * P * P] = np.concatenate(
                        [kb, vb], axis=1).reshape(-1)

                def sprd(a, dt):
                    return a.reshape(n, P, KVH).transpose(1, 2, 0).astype(dt)
                ksb_c[:, :, o: o + n] = sprd(scg, np.float32)
                vsb_c[:, :, o: o + n] = sprd(svg, BF16)
            qt_c[:, s * 32: (s + 1) * 32] = q[b].transpose(1, 0)  # [D, 32]
        in_maps.append(dict(kv=kv_c, ksb=ksb_c.reshape(P, -1),
                            vsb=vsb_c.reshape(P, -1), qt=qt_c))
    return in_maps, padcnt


# ---------------------------------------------------------------------------
# device program
# ---------------------------------------------------------------------------

# cast-engine throughput estimates (free-elems per ns) + per-op fixed ns,
# used only for the static greedy load balancer.  GpSimd is EXCLUDED: it
# shares an SBUF port with the vector engine, and concurrent big gpsimd
# copies collapse both engines to ~1/3 throughput (measured).
CAST_RATE = {"v": 1.92, "s": 1.20}
CAST_FIX = {"v": 170.0, "s": 400.0}


def _build_program(L):
    SLOTS = len(L)
    NTT = sum(L)
    offs = [0]
    for n in L:
        offs.append(offs[-1] + n)
    f32 = mybir.dt.float32
    bf16 = mybir.dt.bfloat16
    i8 = mybir.dt.int8
    EXP = mybir.ActivationFunctionType.Exp

    nc = bacc.Bacc("TRN2", target_bir_lowering=False, debug=False,
                   num_devices=NCORES)

    kv_d = nc.dram_tensor("kv", [1, 16 * NTT * P * P], i8,
                          kind="ExternalInput").ap()
    ksb_d = nc.dram_tensor("ksb", [P, KVH * NTT], f32, kind="ExternalInput").ap()
    vsb_d = nc.dram_tensor("vsb", [P, KVH * NTT], bf16, kind="ExternalInput").ap()
    qt_d = nc.dram_tensor("qt", [P, SLOTS * 32], bf16, kind="ExternalInput").ap()
    po_d = nc.dram_tensor("po", [P, SLOTS * 40], f32, kind="ExternalOutput").ap()

    with tile.TileContext(nc) as tc, ExitStack() as ctx:
        const = ctx.enter_context(tc.tile_pool(name="const", bufs=1))
        p8 = ctx.enter_context(tc.tile_pool(name="p8", bufs=7))
        pb = ctx.enter_context(tc.tile_pool(name="pb", bufs=4))
        work = ctx.enter_context(tc.tile_pool(name="wrk", bufs=5))
        ps_qk = ctx.enter_context(tc.tile_pool(name="psqk", bufs=3, space="PSUM"))
        ps_pt = ctx.enter_context(tc.tile_pool(name="pspt", bufs=2, space="PSUM"))
        ps_pv = ctx.enter_context(tc.tile_pool(name="pspv", bufs=3, space="PSUM"))

        qt = const.tile([P, SLOTS * 32], bf16)
        nc.scalar.dma_start(qt, qt_d)
        ones = const.tile([P, 1], bf16)
        nc.gpsimd.memset(ones, 1.0)
        # prewarm the ACT exp table set so the ~1.3us load is not paid
        # mid-pipeline on the first real exp
        dm = const.tile([P, 1], bf16)
        nc.scalar.activation(dm, ones, EXP)
        # scale vectors for ALL slots in two DMAs on the scalar queue
        # (layout [P, KVH, NTT]: global token-tile index inner)
        ksb_a = const.tile([P, KVH, NTT, 1], f32)
        nc.scalar.dma_start(ksb_a, ksb_d)
        vsb_a = const.tile([P, KVH, NTT, 1], bf16)
        nc.scalar.dma_start(vsb_a, vsb_d)
        # all per-slot outputs accumulate here; ONE out-DMA at the end
        po = const.tile([P, SLOTS * 40], f32)
        nc.gpsimd.memset(po, 0.0)

        # slot order: two medium slots first (enough cast work to cover the
        # big slots' DMA ramp), big slots in the middle, smallest slots last
        # (short serial tail)
        if SLOTS >= 6:
            slot_order = [3, 4] + [0, 1, 2] + list(range(5, SLOTS))
        else:
            slot_order = list(range(SLOTS))
        chunk_list = [(s, jh) for s in slot_order for jh in range(KVH // 2)]

        t8s = {}

        def issue_dma(ci):
            """DMA one (slot, 2-kvh) chunk as raw int8 (sync HWDGE queue)."""
            s, jh = chunk_list[ci]
            n = L[s]
            o = offs[s]
            co = 16 * o * P * P + _chunk_bytes(n) * jh
            t8 = p8.tile([P, 2, 2, n, P], i8, tag="kv8", name="t8")
            nc.sync.dma_start(
                t8, kv_d[0:1, co: co + 4 * n * P * P].rearrange(
                    "o (d r) -> (o d) r", d=P))
            t8s[ci] = t8

        def issue_casts(ci):
            """int8->bf16, rate-matched: DVE takes K + the tail ~25% of V,
            ACT the rest of V -> both engines finish each chunk's cast in
            about the same wall time."""
            s, jh = chunk_list[ci]
            n = L[s]
            t8 = t8s.pop(ci)
            tb = pb.tile([P, 2, 2, n, P], bf16, tag="kvb", name="tb")
            dn = int(round(0.25 * n)) if n >= 8 else 0
            nc.vector.tensor_copy(tb[:, 0], t8[:, 0])            # K on DVE
            if dn > 0:
                nc.vector.tensor_copy(tb[:, 1, :, n - dn:, :],
                                      t8[:, 1, :, n - dn:, :])   # V tail DVE
            nc.scalar.copy(tb[:, 1, :, : n - dn, :],
                           t8[:, 1, :, : n - dn, :])             # V head ACT
            return tb

        slot_state = {}

        def issue_compute(cis, tbs):
            """Compute for a PAIR of chunks (4 kv heads) at once: one
            s1-mul / exp / ev-mul over [P, 4, n, 4] halves the elementwise
            op count (fixed per-op costs dominate these small ops)."""
            s, jh0 = chunk_list[cis[0]]
            n = L[s]
            o = offs[s]
            if jh0 == 0:
                slot_state[s] = (
                    ps_pv.tile([P, 32], f32, tag="pv", name="pv"),
                    ps_pt.tile([4 * n, KVH], f32, tag="pt", name="pt"),
                )
            pv, pt = slot_state[s]
            G = 2 * len(cis)  # kv heads in this group

            qk = ps_qk.tile([P, G, n, 4], f32, tag="qk")
            for ii, ci in enumerate(cis):
                tb = tbs[ii]
                for j2 in range(2):
                    j = 2 * jh0 + 2 * ii + j2
                    qcol = s * 32 + 4 * j
                    for i in range(n):
                        nc.tensor.matmul(
                            qk[:, 2 * ii + j2, i, :],
                            lhsT=tb[:, 0, j2, i, :],
                            rhs=qt[:, qcol: qcol + 4],
                            start=True, stop=True, skip_group_check=True)

            s1 = work.tile([P, G, n, 4], f32, tag="s1")
            nc.vector.tensor_mul(
                s1, qk,
                ksb_a[:, 2 * jh0: 2 * jh0 + G, o: o + n].to_broadcast(
                    [P, G, n, 4]))
            e = work.tile([P, G, n, 4], bf16, tag="e")
            nc.scalar.activation(e, s1, EXP)
            ev = work.tile([P, G, n, 4], bf16, tag="ev")
            nc.vector.tensor_mul(
                ev, e,
                vsb_a[:, 2 * jh0: 2 * jh0 + G, o: o + n].to_broadcast(
                    [P, G, n, 4]))

            for ii, ci in enumerate(cis):
                tb = tbs[ii]
                for j2 in range(2):
                    j = 2 * jh0 + 2 * ii + j2
                    # Z partials: per-(tile, head) column sums of e
                    nc.tensor.matmul(
                        pt[:, j: j + 1],
                        lhsT=e[:, 2 * ii + j2], rhs=ones,
                        start=True, stop=True, skip_group_check=True)
                    # PV accumulate over token tiles: out^T [128d, 4h]
                    cc = 4 * j
                    for i in range(n):
                        nc.tensor.matmul(
                            pv[:, cc: cc + 4],
                            lhsT=tb[:, 1, j2, i, :],
                            rhs=ev[:, 2 * ii + j2, i, :],
                            start=(i == 0), stop=(i == n - 1),
                            skip_group_check=True)

        def issue_slot_end(ci):
            s, jh = chunk_list[ci]
            if jh != KVH // 2 - 1:
                return
            n = L[s]
            pv, pt = slot_state.pop(s)
            # slot done: stage PV and raw Z partials; host does the fold
            nc.vector.tensor_copy(po[:, s * 40: s * 40 + 32], pv)
            nc.scalar.copy(po[0: 4 * n, s * 40 + 32: s * 40 + 40], pt)

        # software-pipelined main loop over PAIRS of chunks.  Per-engine
        # queue order is the expected dependency-ready order (in-order
        # queues): casts for the next chunks (their DMAs ran DMA_AHEAD
        # chunks ahead), then compute for the current pair, then slot-end
        # PSUM drains.
        NCH = len(chunk_list)
        DMA_AHEAD = 7
        for j in range(min(DMA_AHEAD, NCH)):
            issue_dma(j)
        tb_cur = [issue_casts(0), issue_casts(1)]
        for p in range(NCH // 2):
            c0 = 2 * p
            for c in (c0 + DMA_AHEAD, c0 + 1 + DMA_AHEAD):
                if c < NCH:
                    issue_dma(c)
            tb_next = ([issue_casts(c0 + 2), issue_casts(c0 + 3)]
                       if c0 + 2 < NCH else None)
            issue_compute((c0, c0 + 1), tb_cur)
            # drain the PREVIOUS pair's finished slot (its PV/Z are long
            # done, so these PSUM reads never block the cast queue)
            if p > 0:
                issue_slot_end(c0 - 1)
            tb_cur = tb_next
        issue_slot_end(NCH - 1)

        nc.scalar.dma_start(po_d, po)

    nc.compile()
    return nc


_PROGRAM_CACHE = {}


def _get_program(L):
    key = tuple(L)
    if key not in _PROGRAM_CACHE:
        _PROGRAM_CACHE[key] = _build_program(L)
    return _PROGRAM_CACHE[key]


# ---------------------------------------------------------------------------
# entry point
# ---------------------------------------------------------------------------

def kernel(q, k, v, k_cache_q, v_cache_q, k_scale, v_scale,
           block_tables, context_lens, slot_mapping, _trace=False):
    inputs = dict(q=np.asarray(q), k=np.asarray(k), v=np.asarray(v),
                  k_cache_q=np.asarray(k_cache_q),
                  v_cache_q=np.asarray(v_cache_q),
                  k_scale=np.asarray(k_scale), v_scale=np.asarray(v_scale),
                  block_tables=np.asarray(block_tables),
                  context_lens=np.asarray(context_lens),
                  slot_mapping=np.asarray(slot_mapping))
    L, chunks = _plan(inputs["context_lens"])
    in_maps, padcnt = _pack_inputs(inputs, L, chunks)
    nc = _get_program(L)
    res = run_bass_kernel_spmd(nc, in_maps, core_ids=list(range(NCORES)),
                               trace=_trace)

    # combine unnormalized partials across chunks (flash-decoding merge)
    accp = np.zeros((B, P, 32), dtype=np.float64)
    accz = np.zeros((B, 32), dtype=np.float64)
    for c in range(NCORES):
        po = res.results[c]["po"]    # [P, SLOTS*40]
        for s in range(len(L)):
            b, _, _ = chunks[s][c]
            n = L[s]
            accp[b] += po[:, s * 40: s * 40 + 32]
            # raw Z partials [4n, KVH]: row r = tile i*4 + head h
            pt = po[0: 4 * n, s * 40 + 32: s * 40 + 40]
            z32 = pt.reshape(n, 4, KVH).sum(axis=0).T.reshape(32)
            accz[b] += z32 - padcnt[c, s]
    out = (accp / accz[:, None, :]).transpose(0, 2, 1)  # [B, 32h, 128d]
    out = np.ascontiguousarray(out.reshape(B, NUM_HEADS * D), dtype=np.float32)
    if _trace:
        return out, res
    return out


# revision 55
# speedup vs baseline: 1.0002x; 1.0002x over previous
"""Trainium2 Bass kernel: paged int8-KV-cache GQA decode attention, 8-core SPMD.

Contract: kernel(**inputs) takes the FULL unsharded numpy inputs (as produced by
the reference setup_inputs) and returns the FULL [32, 4096] float32 output.

Strategy (data parallel over sequence-chunks, flash-decoding style):
  - The 32 sequences' token tiles (ceil(ctx/128) each) are carved into
    8 cores x SLOTS contiguous chunks; slot s has a fixed tile count L[s]
    shared by all cores (SPMD).  Every chunk computes unnormalized partials
    (PV^T, Z) and the host combines: out = sum(PV) / sum(Z).
  - KV lands in SBUF as RAW INT8 (the DMA engines charge max(src,dst) bytes,
    so int8->int8 halves HBM/DMA time vs the old inline int8->bf16 cast).
    Per (slot, 2-kvh-group) the host packs one contiguous DRAM block whose
    row p is [K(d=p) | V(t=p)], so each chunk is ONE big DMA.
  - On-chip the int8 is cast to bf16 for the matmuls, rate-matched across
    the two usable elementwise engines: DVE (~1.9 fe/ns, 2x_2p mode) takes
    K plus the tail ~25% of V, ACT (~1.1 fe/ns) the rest of V.  GpSimd is
    deliberately unused: it shares an SBUF port with the DVE and concurrent
    big copies collapse both engines ~3x (measured).  Casts are issued one
    chunk ahead of compute (in-order engine queues execute in dependency-
    ready order); chunk DMAs run 6 ahead on the sync HWDGE queue.
  - Per (slot, group of 2 kv heads):
      scores [128t, 2kvh, n, 4h] = per-tile matmuls(lhsT=K^T tile, rhs=q^T)
      s1 = scores * ksb  (DVE; ksb = k_scale*SCALE, zeroed beyond ctx)
      e  = exp(s1) in bf16 (ACT), ev = e * v_scale_vec (DVE)
      Z  = matmul(lhsT=e, rhs=ones) per kvh; pad tokens contribute exp(0)=1,
           corrected host-side via the known count
      PV = matmul(lhsT=V tile, rhs=ev) accumulated in PSUM as out^T [128d,4h]
  Softmax skips max-subtraction (scores are O(20) at most; fp32 exp is safe).
"""

import math
import os
from contextlib import ExitStack

import numpy as np

import sys
sys.path.insert(0, "/opt/trn_rl_repo")

import ml_dtypes  # noqa: E402

import concourse.bass as bass  # noqa: E402
import concourse.mybir as mybir  # noqa: E402
import concourse.tile as tile  # noqa: E402
from concourse import bacc  # noqa: E402
from concourse.bass_utils import run_bass_kernel_spmd  # noqa: E402

BF16 = ml_dtypes.bfloat16

B = 32
NUM_HEADS = 32
KVH = 8
D = 128
REP = NUM_HEADS // KVH  # 4
BLOCK_SIZE = 256
T = 4096
P = 128
SCALE = 1.0 / float(np.sqrt(D))
NCORES = 8

# per-chunk int8 bytes for a (2-kvh, n-tile) group: [K | V] rows
def _chunk_bytes(n):
    return 4 * n * P * P  # (2 kvh) * (K+V) * n tiles * 128 tok * 128 d


# ---------------------------------------------------------------------------
# host-side planning + packing
# ---------------------------------------------------------------------------

def _greedy_chunks(tiles, L):
    """Slot-by-slot, give the 8 largest remaining sequences a chunk of up to
    L[s] tiles.  Returns per-slot lists of (seq, start_tile, len) or None if
    some sequence is left uncovered."""
    rem = [int(t) for t in tiles]
    start = [0] * len(tiles)
    chunks = []
    for Ls in L:
        order = sorted(range(len(tiles)), key=lambda b: -rem[b])
        sc = []
        for c in range(NCORES):
            b = order[c]
            ln = min(rem[b], Ls)
            sc.append((b, start[b], ln))
            rem[b] -= ln
            start[b] += ln
        chunks.append(sc)
    if any(r > 0 for r in rem):
        return None
    return chunks


_PLAN_CACHE = {}


def _plan(context_lens):
    """Choose slot lengths L and the (core, slot) -> sequence-chunk map."""
    tiles = tuple(int(math.ceil(int(c) / P)) for c in context_lens)
    if tiles in _PLAN_CACHE:
        return _PLAN_CACHE[tiles]
    ts = sorted(tiles, reverse=True)
    # octile fallback (always feasible): whole sequences, 4 slots
    best = (ts[0] + ts[8] + ts[16] + ts[24], (ts[0], ts[8], ts[16], ts[24]))
    for L0 in range(max(2, ts[0] - 8), ts[0] + 1):
        for L1 in range(max(2, ts[8] - 8), min(L0, ts[8] + 4) + 1):
            for L2 in range(max(2, ts[16] - 7), min(L1, ts[16] + 4) + 1):
                for L3 in range(max(2, ts[24] - 5), min(L2, ts[24] + 4) + 1):
                    base = L0 + L1 + L2 + L3
                    for L4 in range(2, min(L3, 10) + 1):
                        for L5 in (0, *range(2, L4 + 1)):
                            l6r = (0,) if L5 == 0 else (0, *range(2, L5 + 1))
                            for L6 in l6r:
                                L = tuple(x for x in
                                          (L0, L1, L2, L3, L4, L5, L6) if x)
                                N = sum(L)
                                if N >= best[0]:
                                    continue
                                if _greedy_chunks(tiles, L) is not None:
                                    best = (N, L)
    L = list(best[1])
    chunks = _greedy_chunks(tiles, L)
    _PLAN_CACHE[tiles] = (L, chunks)
    return L, chunks


def _quantize(x):
    absmax = np.abs(x).max(axis=-1)
    scale = np.where(absmax > 0.0, absmax / 127.0, 1.0).astype(np.float32)
    xq = np.clip(np.round(x / scale[..., None]), -127.0, 127.0).astype(np.int32)
    return xq, scale


def _pack_inputs(inputs, L, chunks):
    q = inputs["q"].reshape(B, NUM_HEADS, D).astype(np.float32)
    k = inputs["k"].reshape(B, KVH, D).astype(np.float32)
    v = inputs["v"].reshape(B, KVH, D).astype(np.float32)
    kc = np.ascontiguousarray(inputs["k_cache_q"].reshape(-1, KVH, D))
    vc = np.ascontiguousarray(inputs["v_cache_q"].reshape(-1, KVH, D))
    ks = np.ascontiguousarray(inputs["k_scale"].reshape(-1, KVH)).astype(np.float32)
    vs = np.ascontiguousarray(inputs["v_scale"].reshape(-1, KVH)).astype(np.float32)
    bt = inputs["block_tables"]
    ctx = inputs["context_lens"]
    sm = inputs["slot_mapping"]

    # store_kvcache_int8: quantize the new token and scatter into the cache
    kq, ksn = _quantize(k)
    vq, vsn = _quantize(v)
    kc = kc.copy(); vc = vc.copy(); ks = ks.copy(); vs = vs.copy()
    kc[sm] = kq; vc[sm] = vq; ks[sm] = ksn; vs[sm] = vsn

    SLOTS = len(L)
    NTT = sum(L)
    offs = np.concatenate([[0], np.cumsum(L)])

    in_maps = []
    padcnt = np.zeros((NCORES, SLOTS), dtype=np.float64)
    for c in range(NCORES):
        kv_c = np.zeros((1, 16 * NTT * P * P), dtype=np.int8)
        # scales in global token-tile-major layout [P, KVH, NTT]
        ksb_c = np.zeros((P, KVH, NTT), dtype=np.float32)
        vsb_c = np.zeros((P, KVH, NTT), dtype=BF16)
        qt_c = np.zeros((P, SLOTS * 32), dtype=BF16)
        for s in range(SLOTS):
            b, t0, ln = chunks[s][c]
            n = L[s]
            nt = n * P
            o = int(offs[s])
            nvalid = max(0, min(int(ctx[b]) - t0 * P, ln * P))
            padcnt[c, s] = nt - nvalid
            if ln > 0:
                flat = (bt[b][:, None] * BLOCK_SIZE
                        + np.arange(BLOCK_SIZE, dtype=np.int64)[None, :]
                        ).reshape(-1)[t0 * P: t0 * P + ln * P]
                kg = np.zeros((nt, KVH, D), dtype=np.int8)
                vg = np.zeros((nt, KVH, D), dtype=np.int8)
                kg[: ln * P] = kc[flat]
                vg[: ln * P] = vc[flat]
                scg = np.zeros((nt, KVH), dtype=np.float32)
                svg = np.zeros((nt, KVH), dtype=np.float32)
                valid = (np.arange(nt) < nvalid)
                scg[: ln * P] = ks[flat] * SCALE
                svg[: ln * P] = vs[flat]
                scg *= valid[:, None]
                svg *= valid[:, None]
                kjdt = kg.transpose(1, 2, 0)                      # [KVH, D, nt]
                vpjid = vg.reshape(n, P, KVH, D).transpose(1, 2, 0, 3)
                for jh in range(KVH // 2):
                    co = 16 * o * P * P + _chunk_bytes(n) * jh
                    kb = kjdt[2 * jh: 2 * jh + 2].transpose(1, 0, 2
                                                            ).reshape(P, -1)
                    vb = vpjid[:, 2 * jh: 2 * jh + 2].reshape(P, -1)
                    kv_c[0, co: co + 4 * n * P * P] = np.concatenate(
                        [kb, vb], axis=1).reshape(-1)

                def sprd(a, dt):
                    return a.reshape(n, P, KVH).transpose(1, 2, 0).astype(dt)
                ksb_c[:, :, o: o + n] = sprd(scg, np.float32)
                vsb_c[:, :, o: o + n] = sprd(svg, BF16)
            qt_c[:, s * 32: (s + 1) * 32] = q[b].transpose(1, 0)  # [D, 32]
        in_maps.append(dict(kv=kv_c, ksb=ksb_c.reshape(P, -1),
                            vsb=vsb_c.reshape(P, -1), qt=qt_c))
    return in_maps, padcnt


# ---------------------------------------------------------------------------
# device program
# ---------------------------------------------------------------------------

# cast-engine throughput estimates (free-elems per ns) + per-op fixed ns,
# used only for the static greedy load balancer.  GpSimd is EXCLUDED: it
# shares an SBUF port with the vector engine, and concurrent big gpsimd
# copies collapse both engines to ~1/3 throughput (measured).
CAST_RATE = {"v": 1.92, "s": 1.20}
CAST_FIX = {"v": 170.0, "s": 400.0}


def _build_program(L):
    SLOTS = len(L)
    NTT = sum(L)
    offs = [0]
    for n in L:
        offs.append(offs[-1] + n)
    f32 = mybir.dt.float32
    bf16 = mybir.dt.bfloat16
    i8 = mybir.dt.int8
    EXP = mybir.ActivationFunctionType.Exp

    nc = bacc.Bacc("TRN2", target_bir_lowering=False, debug=False,
                   num_devices=NCORES)

    kv_d = nc.dram_tensor("kv", [1, 16 * NTT * P * P], i8,
                          kind="ExternalInput").ap()
    ksb_d = nc.dram_tensor("ksb", [P, KVH * NTT], f32, kind="ExternalInput").ap()
    vsb_d = nc.dram_tensor("vsb", [P, KVH * NTT], bf16, kind="ExternalInput").ap()
    qt_d = nc.dram_tensor("qt", [P, SLOTS * 32], bf16, kind="ExternalInput").ap()
    po_d = nc.dram_tensor("po", [P, SLOTS * 40], f32, kind="ExternalOutput").ap()

    with tile.TileContext(nc) as tc, ExitStack() as ctx:
        const = ctx.enter_context(tc.tile_pool(name="const", bufs=1))
        p8 = ctx.enter_context(tc.tile_pool(name="p8", bufs=7))
        pb = ctx.enter_context(tc.tile_pool(name="pb", bufs=4))
        work = ctx.enter_context(tc.tile_pool(name="wrk", bufs=5))
        ps_qk = ctx.enter_context(tc.tile_pool(name="psqk", bufs=3, space="PSUM"))
        ps_pt = ctx.enter_context(tc.tile_pool(name="pspt", bufs=2, space="PSUM"))
        ps_pv = ctx.enter_context(tc.tile_pool(name="pspv", bufs=3, space="PSUM"))

        qt = const.tile([P, SLOTS * 32], bf16)
        nc.scalar.dma_start(qt, qt_d)
        ones = const.tile([P, 1], bf16)
        nc.gpsimd.memset(ones, 1.0)
        # prewarm the ACT exp table set so the ~1.3us load is not paid
        # mid-pipeline on the first real exp
        dm = const.tile([P, 1], bf16)
        nc.scalar.activation(dm, ones, EXP)
        # scale vectors for ALL slots in two DMAs on the scalar queue
        # (layout [P, KVH, NTT]: global token-tile index inner)
        ksb_a = const.tile([P, KVH, NTT, 1], f32)
        nc.scalar.dma_start(ksb_a, ksb_d)
        vsb_a = const.tile([P, KVH, NTT, 1], bf16)
        nc.scalar.dma_start(vsb_a, vsb_d)
        # all per-slot outputs accumulate here; ONE out-DMA at the end
        po = const.tile([P, SLOTS * 40], f32)
        nc.gpsimd.memset(po, 0.0)

        # slot order: two medium slots first (enough cast work to cover the
        # big slots' DMA ramp), big slots in the middle, smallest slots last
        # (short serial tail)
        if SLOTS >= 6:
            slot_order = [3, 4] + [0, 1, 2] + list(range(5, SLOTS))
        else:
            slot_order = list(range(SLOTS))
        chunk_list = [(s, jh) for s in slot_order for jh in range(KVH // 2)]

        t8s = {}

        def issue_dma(ci):
            """DMA one (slot, 2-kvh) chunk as raw int8 (sync HWDGE queue)."""
            s, jh = chunk_list[ci]
            n = L[s]
            o = offs[s]
            co = 16 * o * P * P + _chunk_bytes(n) * jh
            t8 = p8.tile([P, 2, 2, n, P], i8, tag="kv8", name="t8")
            nc.sync.dma_start(
                t8, kv_d[0:1, co: co + 4 * n * P * P].rearrange(
                    "o (d r) -> (o d) r", d=P))
            t8s[ci] = t8

        def issue_casts(ci):
            """int8->bf16, rate-matched: DVE takes K + the tail ~25% of V,
            ACT the rest of V -> both engines finish each chunk's cast in
            about the same wall time."""
            s, jh = chunk_list[ci]
            n = L[s]
            t8 = t8s.pop(ci)
            tb = pb.tile([P, 2, 2, n, P], bf16, tag="kvb", name="tb")
            dn = int(round(0.25 * n)) if n >= 8 else 0
            nc.vector.tensor_copy(tb[:, 0], t8[:, 0])            # K on DVE
            if dn > 0:
                nc.vector.tensor_copy(tb[:, 1, :, n - dn:, :],
                                      t8[:, 1, :, n - dn:, :])   # V tail DVE
            nc.scalar.copy(tb[:, 1, :, : n - dn, :],
                           t8[:, 1, :, : n - dn, :])             # V head ACT
            return tb

        slot_state = {}

        def issue_compute(cis, tbs):
            """Compute for a PAIR of chunks (4 kv heads) at once: one
            s1-mul / exp / ev-mul over [P, 4, n, 4] halves the elementwise
            op count (fixed per-op costs dominate these small ops)."""
            s, jh0 = chunk_list[cis[0]]
            n = L[s]
            o = offs[s]
            if jh0 == 0:
                slot_state[s] = (
                    ps_pv.tile([P, 32], f32, tag="pv", name="pv"),
                    ps_pt.tile([4 * n, KVH], f32, tag="pt", name="pt"),
                )
            pv, pt = slot_state[s]
            G = 2 * len(cis)  # kv heads in this group

            qk = ps_qk.tile([P, G, n, 4], f32, tag="qk")
            for ii, ci in enumerate(cis):
                tb = tbs[ii]
                for j2 in range(2):
                    j = 2 * jh0 + 2 * ii + j2
                    qcol = s * 32 + 4 * j
                    for i in range(n):
                        nc.tensor.matmul(
                            qk[:, 2 * ii + j2, i, :],
                            lhsT=tb[:, 0, j2, i, :],
                            rhs=qt[:, qcol: qcol + 4],
                            start=True, stop=True, skip_group_check=True)

            s1 = work.tile([P, G, n, 4], f32, tag="s1")
            nc.vector.tensor_mul(
                s1, qk,
                ksb_a[:, 2 * jh0: 2 * jh0 + G, o: o + n].to_broadcast(
                    [P, G, n, 4]))
            e = work.tile([P, G, n, 4], bf16, tag="e")
            nc.scalar.activation(e, s1, EXP)
            ev = work.tile([P, G, n, 4], bf16, tag="ev")
            nc.vector.tensor_mul(
                ev, e,
                vsb_a[:, 2 * jh0: 2 * jh0 + G, o: o + n].to_broadcast(
                    [P, G, n, 4]))

            for ii, ci in enumerate(cis):
                tb = tbs[ii]
                for j2 in range(2):
                    j = 2 * jh0 + 2 * ii + j2
                    # Z partials: per-(tile, head) column sums of e
                    nc.tensor.matmul(
                        pt[:, j: j + 1],
                        lhsT=e[:, 2 * ii + j2], rhs=ones,
                        start=True, stop=True, skip_group_check=True)
                    # PV accumulate over token tiles: out^T [128d, 4h]
                    cc = 4 * j
                    for i in range(n):
                        nc.tensor.matmul(
                            pv[:, cc: cc + 4],
                            lhsT=tb[:, 1, j2, i, :],
                            rhs=ev[:, 2 * ii + j2, i, :],
                            start=(i == 0), stop=(i == n - 1),
                            skip_group_check=True)

        def issue_slot_end(ci):
            s, jh = chunk_list[ci]
            if jh != KVH // 2 - 1:
                return
            n = L[s]
            pv, pt = slot_state.pop(s)
            # slot done: stage PV and raw Z partials; host does the fold
            nc.vector.tensor_copy(po[:, s * 40: s * 40 + 32], pv)
            nc.scalar.copy(po[0: 4 * n, s * 40 + 32: s * 40 + 40], pt)

        # software-pipelined main loop over PAIRS of chunks.  Per-engine
        # queue order is the expected dependency-ready order (in-order
        # queues): casts for the next chunks (their DMAs ran DMA_AHEAD
        # chunks ahead), then compute for the current pair, then slot-end
        # PSUM drains.
        NCH = len(chunk_list)
        DMA_AHEAD = 7
        for j in range(min(DMA_AHEAD, NCH)):
            issue_dma(j)
        tb_cur = [issue_casts(0), issue_casts(1)]
        for p in range(NCH // 2):
            c0 = 2 * p
            for c in (c0 + DMA_AHEAD, c0 + 1 + DMA_AHEAD):
                if c < NCH:
                    issue_dma(c)
            tb_next = ([issue_casts(c0 + 2), issue_casts(c0 + 3)]
                       if c0 + 2 < NCH else None)
            issue_compute((c0, c0 + 1), tb_cur)
            # drain the PREVIOUS pair's finished slot (its PV/Z are long
            # done, so these PSUM reads never block the cast queue)
            if p > 0:
                issue_slot_end(c0 - 1)
            tb_cur = tb_next
        issue_slot_end(NCH - 1)

        nc.scalar.dma_start(po_d, po)

    nc.compile()
    return nc


_PROGRAM_CACHE = {}


def _get_program(L):
    key = tuple(L)
    if key not in _PROGRAM_CACHE:
        _PROGRAM_CACHE[key] = _build_program(L)
    return _PROGRAM_CACHE[key]


# ---------------------------------------------------------------------------
# entry point
# ---------------------------------------------------------------------------

def kernel(q, k, v, k_cache_q, v_cache_q, k_scale, v_scale,
           block_tables, context_lens, slot_mapping, _trace=False):
    inputs = dict(q=np.asarray(q), k=np.asarray(k), v=np.asarray(v),
                  k_cache_q=np.asarray(k_cache_q),
                  v_cache_q=np.asarray(v_cache_q),
                  k_scale=np.asarray(k_scale), v_scale=np.asarray(v_scale),
                  block_tables=np.asarray(block_tables),
                  context_lens=np.asarray(context_lens),
                  slot_mapping=np.asarray(slot_mapping))
    L, chunks = _plan(inputs["context_lens"])
    in_maps, padcnt = _pack_inputs(inputs, L, chunks)
    nc = _get_program(L)
    res = run_bass_kernel_spmd(nc, in_maps, core_ids=list(range(NCORES)),
                               trace=_trace)

    # combine unnormalized partials across chunks (flash-decoding merge)
    accp = np.zeros((B, P, 32), dtype=np.float64)
    accz = np.zeros((B, 32), dtype=np.float64)
    for c in range(NCORES):
        po = res.results[c]["po"]    # [P, SLOTS*40]
        for s in range(len(L)):
            b, _, _ = chunks[s][c]
            n = L[s]
            accp[b] += po[:, s * 40: s * 40 + 32]
            # raw Z partials [4n, KVH]: row r = tile i*4 + head h
            pt = po[0: 4 * n, s * 40 + 32: s * 40 + 40]
            z32 = pt.reshape(n, 4, KVH).sum(axis=0).T.reshape(32)
            accz[b] += z32 - padcnt[c, s]
    out = (accp / accz[:, None, :]).transpose(0, 2, 1)  # [B, 32h, 128d]
    out = np.ascontiguousarray(out.reshape(B, NUM_HEADS * D), dtype=np.float32)
    if _trace:
        return out, res
    return out


# revision 56
# speedup vs baseline: 1.1375x; 1.1373x over previous
"""Trainium2 Bass kernel: paged int8-KV-cache GQA decode attention, 8-core SPMD.

Contract: kernel(**inputs) takes the FULL unsharded numpy inputs (as produced by
the reference setup_inputs) and returns the FULL [32, 4096] float32 output.

Strategy (data parallel over sequence-chunks, flash-decoding style):
  - The 32 sequences' token tiles (ceil(ctx/128) each) are carved into
    8 cores x SLOTS contiguous chunks; slot s has a fixed tile count L[s]
    shared by all cores (SPMD).  Every chunk computes unnormalized partials
    (PV^T, Z) and the host combines: out = sum(PV) / sum(Z).
  - KV lands in SBUF as RAW INT8 (the DMA engines charge max(src,dst) bytes,
    so int8->int8 halves HBM/DMA time vs the old inline int8->bf16 cast).
    Per (slot, 2-kvh-group) the host packs one contiguous DRAM block whose
    row p is [K(d=p) | V(t=p)], so each chunk is ONE big DMA.
  - On-chip the int8 is cast to bf16 for the matmuls, rate-matched across
    the two usable elementwise engines: DVE (~1.9 fe/ns, 2x_2p mode) takes
    K plus the tail ~25% of V, ACT (~1.1 fe/ns) the rest of V.  GpSimd is
    deliberately unused: it shares an SBUF port with the DVE and concurrent
    big copies collapse both engines ~3x (measured).  Casts are issued one
    chunk ahead of compute (in-order engine queues execute in dependency-
    ready order); chunk DMAs run 6 ahead on the sync HWDGE queue.
  - Per (slot, group of 2 kv heads):
      scores [128t, 2kvh, n, 4h] = per-tile matmuls(lhsT=K^T tile, rhs=q^T)
      s1 = scores * ksb  (DVE; ksb = k_scale*SCALE, zeroed beyond ctx)
      e  = exp(s1) in bf16 (ACT), ev = e * v_scale_vec (DVE)
      Z  = matmul(lhsT=e, rhs=ones) per kvh; pad tokens contribute exp(0)=1,
           corrected host-side via the known count
      PV = matmul(lhsT=V tile, rhs=ev) accumulated in PSUM as out^T [128d,4h]
  Softmax skips max-subtraction (scores are O(20) at most; fp32 exp is safe).
"""

import math
import os
from contextlib import ExitStack

import numpy as np

import sys
sys.path.insert(0, "/opt/trn_rl_repo")

import ml_dtypes  # noqa: E402

import concourse.bass as bass  # noqa: E402
import concourse.mybir as mybir  # noqa: E402
import concourse.tile as tile  # noqa: E402
from concourse import bacc  # noqa: E402
from concourse.bass_utils import run_bass_kernel_spmd  # noqa: E402

BF16 = ml_dtypes.bfloat16

B = 32
NUM_HEADS = 32
KVH = 8
D = 128
REP = NUM_HEADS // KVH  # 4
BLOCK_SIZE = 256
T = 4096
P = 128
SCALE = 1.0 / float(np.sqrt(D))
NCORES = 8

# per-chunk int8 bytes for a (2-kvh, n-tile) group: [K | V] rows
def _chunk_bytes(n):
    return 4 * n * P * P  # (2 kvh) * (K+V) * n tiles * 128 tok * 128 d


# ---------------------------------------------------------------------------
# host-side planning + packing
# ---------------------------------------------------------------------------

def _greedy_chunks(tiles, L):
    """Slot-by-slot, give the 8 largest remaining sequences a chunk of up to
    L[s] tiles.  Returns per-slot lists of (seq, start_tile, len) or None if
    some sequence is left uncovered."""
    rem = [int(t) for t in tiles]
    start = [0] * len(tiles)
    chunks = []
    for Ls in L:
        order = sorted(range(len(tiles)), key=lambda b: -rem[b])
        sc = []
        for c in range(NCORES):
            b = order[c]
            ln = min(rem[b], Ls)
            sc.append((b, start[b], ln))
            rem[b] -= ln
            start[b] += ln
        chunks.append(sc)
    if any(r > 0 for r in rem):
        return None
    return chunks


_PLAN_CACHE = {}


def _plan(context_lens):
    """Choose slot lengths L and the (core, slot) -> sequence-chunk map."""
    tiles = tuple(int(math.ceil(int(c) / P)) for c in context_lens)
    if tiles in _PLAN_CACHE:
        return _PLAN_CACHE[tiles]
    ts = sorted(tiles, reverse=True)
    # octile fallback (always feasible): whole sequences, 4 slots
    best = (ts[0] + ts[8] + ts[16] + ts[24], (ts[0], ts[8], ts[16], ts[24]))
    for L0 in range(max(2, ts[0] - 8), ts[0] + 1):
        for L1 in range(max(2, ts[8] - 8), min(L0, ts[8] + 4) + 1):
            for L2 in range(max(2, ts[16] - 7), min(L1, ts[16] + 4) + 1):
                for L3 in range(max(2, ts[24] - 5), min(L2, ts[24] + 4) + 1):
                    base = L0 + L1 + L2 + L3
                    for L4 in range(2, min(L3, 10) + 1):
                        for L5 in (0, *range(2, L4 + 1)):
                            l6r = (0,) if L5 == 0 else (0, *range(2, L5 + 1))
                            for L6 in l6r:
                                L = tuple(x for x in
                                          (L0, L1, L2, L3, L4, L5, L6) if x)
                                N = sum(L)
                                if N >= best[0]:
                                    continue
                                if _greedy_chunks(tiles, L) is not None:
                                    best = (N, L)
    L = list(best[1])
    chunks = _greedy_chunks(tiles, L)
    _PLAN_CACHE[tiles] = (L, chunks)
    return L, chunks


def _quantize(x):
    absmax = np.abs(x).max(axis=-1)
    scale = np.where(absmax > 0.0, absmax / 127.0, 1.0).astype(np.float32)
    xq = np.clip(np.round(x / scale[..., None]), -127.0, 127.0).astype(np.int32)
    return xq, scale


def _pack_inputs(inputs, L, chunks):
    q = inputs["q"].reshape(B, NUM_HEADS, D).astype(np.float32)
    k = inputs["k"].reshape(B, KVH, D).astype(np.float32)
    v = inputs["v"].reshape(B, KVH, D).astype(np.float32)
    kc = np.ascontiguousarray(inputs["k_cache_q"].reshape(-1, KVH, D))
    vc = np.ascontiguousarray(inputs["v_cache_q"].reshape(-1, KVH, D))
    ks = np.ascontiguousarray(inputs["k_scale"].reshape(-1, KVH)).astype(np.float32)
    vs = np.ascontiguousarray(inputs["v_scale"].reshape(-1, KVH)).astype(np.float32)
    bt = inputs["block_tables"]
    ctx = inputs["context_lens"]
    sm = inputs["slot_mapping"]

    # store_kvcache_int8: quantize the new token and scatter into the cache
    kq, ksn = _quantize(k)
    vq, vsn = _quantize(v)
    kc = kc.copy(); vc = vc.copy(); ks = ks.copy(); vs = vs.copy()
    kc[sm] = kq; vc[sm] = vq; ks[sm] = ksn; vs[sm] = vsn

    SLOTS = len(L)
    NTT = sum(L)
    offs = np.concatenate([[0], np.cumsum(L)])

    in_maps = []
    padcnt = np.zeros((NCORES, SLOTS), dtype=np.float64)
    for c in range(NCORES):
        kv_c = np.zeros((1, 16 * NTT * P * P), dtype=np.int8)
        # scales in global token-tile-major layout [P, KVH, NTT]
        ksb_c = np.zeros((P, KVH, NTT), dtype=np.float32)
        vsb_c = np.zeros((P, KVH, NTT), dtype=BF16)
        qt_c = np.zeros((P, SLOTS * 32), dtype=BF16)
        for s in range(SLOTS):
            b, t0, ln = chunks[s][c]
            n = L[s]
            nt = n * P
            o = int(offs[s])
            nvalid = max(0, min(int(ctx[b]) - t0 * P, ln * P))
            padcnt[c, s] = nt - nvalid
            if ln > 0:
                flat = (bt[b][:, None] * BLOCK_SIZE
                        + np.arange(BLOCK_SIZE, dtype=np.int64)[None, :]
                        ).reshape(-1)[t0 * P: t0 * P + ln * P]
                kg = np.zeros((nt, KVH, D), dtype=np.int8)
                vg = np.zeros((nt, KVH, D), dtype=np.int8)
                kg[: ln * P] = kc[flat]
                vg[: ln * P] = vc[flat]
                scg = np.zeros((nt, KVH), dtype=np.float32)
                svg = np.zeros((nt, KVH), dtype=np.float32)
                valid = (np.arange(nt) < nvalid)
                scg[: ln * P] = ks[flat] * SCALE
                svg[: ln * P] = vs[flat]
                scg *= valid[:, None]
                svg *= valid[:, None]
                kjdt = kg.transpose(1, 2, 0)                      # [KVH, D, nt]
                vpjid = vg.reshape(n, P, KVH, D).transpose(1, 2, 0, 3)
                for jh in range(KVH // 2):
                    co = 16 * o * P * P + _chunk_bytes(n) * jh
                    kb = kjdt[2 * jh: 2 * jh + 2].transpose(1, 0, 2
                                                            ).reshape(P, -1)
                    vb = vpjid[:, 2 * jh: 2 * jh + 2].reshape(P, -1)
                    kv_c[0, co: co + 4 * n * P * P] = np.concatenate(
                        [kb, vb], axis=1).reshape(-1)

                def sprd(a, dt):
                    return a.reshape(n, P, KVH).transpose(1, 2, 0).astype(dt)
                ksb_c[:, :, o: o + n] = sprd(scg, np.float32)
                vsb_c[:, :, o: o + n] = sprd(svg, BF16)
            qt_c[:, s * 32: (s + 1) * 32] = q[b].transpose(1, 0)  # [D, 32]
        in_maps.append(dict(kv=kv_c, ksb=ksb_c.reshape(P, -1),
                            vsb=vsb_c.reshape(P, -1), qt=qt_c))
    return in_maps, padcnt


# ---------------------------------------------------------------------------
# device program
# ---------------------------------------------------------------------------

# cast-engine throughput estimates (free-elems per ns) + per-op fixed ns,
# used only for the static greedy load balancer.  GpSimd is EXCLUDED: it
# shares an SBUF port with the vector engine, and concurrent big gpsimd
# copies collapse both engines to ~1/3 throughput (measured).
CAST_RATE = {"v": 1.92, "s": 1.20}
CAST_FIX = {"v": 170.0, "s": 400.0}


def _build_program(L):
    SLOTS = len(L)
    NTT = sum(L)
    offs = [0]
    for n in L:
        offs.append(offs[-1] + n)
    f32 = mybir.dt.float32
    bf16 = mybir.dt.bfloat16
    i8 = mybir.dt.int8
    EXP = mybir.ActivationFunctionType.Exp

    nc = bacc.Bacc("TRN2", target_bir_lowering=False, debug=False,
                   num_devices=NCORES)

    kv_d = nc.dram_tensor("kv", [1, 16 * NTT * P * P], i8,
                          kind="ExternalInput").ap()
    ksb_d = nc.dram_tensor("ksb", [P, KVH * NTT], f32, kind="ExternalInput").ap()
    vsb_d = nc.dram_tensor("vsb", [P, KVH * NTT], bf16, kind="ExternalInput").ap()
    qt_d = nc.dram_tensor("qt", [P, SLOTS * 32], bf16, kind="ExternalInput").ap()
    po_d = nc.dram_tensor("po", [P, SLOTS * 40], f32, kind="ExternalOutput").ap()

    with tile.TileContext(nc) as tc, ExitStack() as ctx:
        const = ctx.enter_context(tc.tile_pool(name="const", bufs=1))
        p8 = ctx.enter_context(tc.tile_pool(name="p8", bufs=7))
        pb = ctx.enter_context(tc.tile_pool(name="pb", bufs=4))
        work = ctx.enter_context(tc.tile_pool(name="wrk", bufs=5))
        ps_qk = ctx.enter_context(tc.tile_pool(name="psqk", bufs=3, space="PSUM"))
        ps_pt = ctx.enter_context(tc.tile_pool(name="pspt", bufs=2, space="PSUM"))
        ps_pv = ctx.enter_context(tc.tile_pool(name="pspv", bufs=3, space="PSUM"))

        qt = const.tile([P, SLOTS * 32], bf16)
        nc.scalar.dma_start(qt, qt_d)
        ones = const.tile([P, 1], bf16)
        nc.gpsimd.memset(ones, 1.0)
        # prewarm the ACT exp table set so the ~1.3us load is not paid
        # mid-pipeline on the first real exp
        dm = const.tile([P, 1], bf16)
        nc.scalar.activation(dm, ones, EXP)
        # scale vectors for ALL slots in two DMAs on the scalar queue
        # (layout [P, KVH, NTT]: global token-tile index inner)
        ksb_a = const.tile([P, KVH, NTT, 1], f32)
        nc.scalar.dma_start(ksb_a, ksb_d)
        vsb_a = const.tile([P, KVH, NTT, 1], bf16)
        nc.scalar.dma_start(vsb_a, vsb_d)
        # all per-slot outputs accumulate here; ONE out-DMA at the end
        po = const.tile([P, SLOTS * 40], f32)
        nc.gpsimd.memset(po, 0.0)

        # slot order: two medium slots first (enough cast work to cover the
        # big slots' DMA ramp), big slots in the middle, smallest slots last
        # (short serial tail)
        if SLOTS >= 6:
            slot_order = [3, 4] + [0, 1, 2] + list(range(5, SLOTS))
        else:
            slot_order = list(range(SLOTS))
        chunk_list = [(s, jh) for s in slot_order for jh in range(KVH // 2)]

        t8s = {}

        def issue_dma(ci):
            """DMA one (slot, 2-kvh) chunk as raw int8 (sync HWDGE queue)."""
            s, jh = chunk_list[ci]
            n = L[s]
            o = offs[s]
            co = 16 * o * P * P + _chunk_bytes(n) * jh
            t8 = p8.tile([P, 2, 2, n, P], i8, tag="kv8", name="t8")
            nc.sync.dma_start(
                t8, kv_d[0:1, co: co + 4 * n * P * P].rearrange(
                    "o (d r) -> (o d) r", d=P))
            t8s[ci] = t8

        def issue_casts(ci):
            """int8->bf16, rate-matched: DVE takes K + the tail ~25% of V,
            ACT the rest of V -> both engines finish each chunk's cast in
            about the same wall time."""
            s, jh = chunk_list[ci]
            n = L[s]
            t8 = t8s.pop(ci)
            tb = pb.tile([P, 2, 2, n, P], bf16, tag="kvb", name="tb")
            dn = int(round(0.28 * n)) if n >= 8 else 0
            nc.vector.tensor_copy(tb[:, 0], t8[:, 0])            # K on DVE
            if dn > 0:
                nc.vector.tensor_copy(tb[:, 1, :, n - dn:, :],
                                      t8[:, 1, :, n - dn:, :])   # V tail DVE
            nc.scalar.copy(tb[:, 1, :, : n - dn, :],
                           t8[:, 1, :, : n - dn, :])             # V head ACT
            return tb

        slot_state = {}

        def issue_compute(cis, tbs):
            """Compute for a PAIR of chunks (4 kv heads) at once: one
            s1-mul / exp / ev-mul over [P, 4, n, 4] halves the elementwise
            op count (fixed per-op costs dominate these small ops)."""
            s, jh0 = chunk_list[cis[0]]
            n = L[s]
            o = offs[s]
            if jh0 == 0:
                slot_state[s] = (
                    ps_pv.tile([P, 32], f32, tag="pv", name="pv"),
                    ps_pt.tile([4 * n, KVH], f32, tag="pt", name="pt"),
                )
            pv, pt = slot_state[s]
            G = 2 * len(cis)  # kv heads in this group

            qk = ps_qk.tile([P, G, n, 4], f32, tag="qk")
            for ii, ci in enumerate(cis):
                tb = tbs[ii]
                for j2 in range(2):
                    j = 2 * jh0 + 2 * ii + j2
                    qcol = s * 32 + 4 * j
                    for i in range(n):
                        nc.tensor.matmul(
                            qk[:, 2 * ii + j2, i, :],
                            lhsT=tb[:, 0, j2, i, :],
                            rhs=qt[:, qcol: qcol + 4],
                            start=True, stop=True, skip_group_check=True)

            s1 = work.tile([P, G, n, 4], f32, tag="s1")
            nc.vector.tensor_mul(
                s1, qk,
                ksb_a[:, 2 * jh0: 2 * jh0 + G, o: o + n].to_broadcast(
                    [P, G, n, 4]))
            e = work.tile([P, G, n, 4], bf16, tag="e")
            nc.scalar.activation(e, s1, EXP)
            ev = work.tile([P, G, n, 4], bf16, tag="ev")
            nc.vector.tensor_mul(
                ev, e,
                vsb_a[:, 2 * jh0: 2 * jh0 + G, o: o + n].to_broadcast(
                    [P, G, n, 4]))

            for ii, ci in enumerate(cis):
                tb = tbs[ii]
                for j2 in range(2):
                    j = 2 * jh0 + 2 * ii + j2
                    # Z partials: per-(tile, head) column sums of e
                    nc.tensor.matmul(
                        pt[:, j: j + 1],
                        lhsT=e[:, 2 * ii + j2], rhs=ones,
                        start=True, stop=True, skip_group_check=True)
                    # PV accumulate over token tiles: out^T [128d, 4h]
                    cc = 4 * j
                    for i in range(n):
                        nc.tensor.matmul(
                            pv[:, cc: cc + 4],
                            lhsT=tb[:, 1, j2, i, :],
                            rhs=ev[:, 2 * ii + j2, i, :],
                            start=(i == 0), stop=(i == n - 1),
                            skip_group_check=True)

        def issue_slot_end(ci):
            s, jh = chunk_list[ci]
            if jh != KVH // 2 - 1:
                return
            n = L[s]
            pv, pt = slot_state.pop(s)
            # slot done: stage PV and raw Z partials; host does the fold
            nc.vector.tensor_copy(po[:, s * 40: s * 40 + 32], pv)
            nc.scalar.copy(po[0: 4 * n, s * 40 + 32: s * 40 + 40], pt)

        # software-pipelined main loop over PAIRS of chunks.  Per-engine
        # queue order is the expected dependency-ready order (in-order
        # queues): casts for the next chunks (their DMAs ran DMA_AHEAD
        # chunks ahead), then compute for the current pair, then slot-end
        # PSUM drains.
        NCH = len(chunk_list)
        DMA_AHEAD = 7
        for j in range(min(DMA_AHEAD, NCH)):
            issue_dma(j)
        tb_cur = [issue_casts(0), issue_casts(1)]
        for p in range(NCH // 2):
            c0 = 2 * p
            for c in (c0 + DMA_AHEAD, c0 + 1 + DMA_AHEAD):
                if c < NCH:
                    issue_dma(c)
            tb_next = ([issue_casts(c0 + 2), issue_casts(c0 + 3)]
                       if c0 + 2 < NCH else None)
            issue_compute((c0, c0 + 1), tb_cur)
            # drain the PREVIOUS pair's finished slot (its PV/Z are long
            # done, so these PSUM reads never block the cast queue)
            if p > 0:
                issue_slot_end(c0 - 1)
            tb_cur = tb_next
        issue_slot_end(NCH - 1)

        nc.scalar.dma_start(po_d, po)

    nc.compile()
    return nc


_PROGRAM_CACHE = {}


def _get_program(L):
    key = tuple(L)
    if key not in _PROGRAM_CACHE:
        _PROGRAM_CACHE[key] = _build_program(L)
    return _PROGRAM_CACHE[key]


# ---------------------------------------------------------------------------
# entry point
# ---------------------------------------------------------------------------

def kernel(q, k, v, k_cache_q, v_cache_q, k_scale, v_scale,
           block_tables, context_lens, slot_mapping, _trace=False):
    inputs = dict(q=np.asarray(q), k=np.asarray(k), v=np.asarray(v),
                  k_cache_q=np.asarray(k_cache_q),
                  v_cache_q=np.asarray(v_cache_q),
                  k_scale=np.asarray(k_scale), v_scale=np.asarray(v_scale),
                  block_tables=np.asarray(block_tables),
                  context_lens=np.asarray(context_lens),
                  slot_mapping=np.asarray(slot_mapping))
    L, chunks = _plan(inputs["context_lens"])
    in_maps, padcnt = _pack_inputs(inputs, L, chunks)
    nc = _get_program(L)
    res = run_bass_kernel_spmd(nc, in_maps, core_ids=list(range(NCORES)),
                               trace=_trace)

    # combine unnormalized partials across chunks (flash-decoding merge)
    accp = np.zeros((B, P, 32), dtype=np.float64)
    accz = np.zeros((B, 32), dtype=np.float64)
    for c in range(NCORES):
        po = res.results[c]["po"]    # [P, SLOTS*40]
        for s in range(len(L)):
            b, _, _ = chunks[s][c]
            n = L[s]
            accp[b] += po[:, s * 40: s * 40 + 32]
            # raw Z partials [4n, KVH]: row r = tile i*4 + head h
            pt = po[0: 4 * n, s * 40 + 32: s * 40 + 40]
            z32 = pt.reshape(n, 4, KVH).sum(axis=0).T.reshape(32)
            accz[b] += z32 - padcnt[c, s]
    out = (accp / accz[:, None, :]).transpose(0, 2, 1)  # [B, 32h, 128d]
    out = np.ascontiguousarray(out.reshape(B, NUM_HEADS * D), dtype=np.float32)
    if _trace:
        return out, res
    return out


# revision 57
# speedup vs baseline: 1.1924x; 1.0483x over previous
"""Trainium2 Bass kernel: paged int8-KV-cache GQA decode attention, 8-core SPMD.

Contract: kernel(**inputs) takes the FULL unsharded numpy inputs (as produced by
the reference setup_inputs) and returns the FULL [32, 4096] float32 output.

Strategy (data parallel over sequence-chunks, flash-decoding style):
  - The 32 sequences' token tiles (ceil(ctx/128) each) are carved into
    8 cores x SLOTS contiguous chunks; slot s has a fixed tile count L[s]
    shared by all cores (SPMD).  Every chunk computes unnormalized partials
    (PV^T, Z) and the host combines: out = sum(PV) / sum(Z).
  - KV lands in SBUF as RAW INT8 (the DMA engines charge max(src,dst) bytes,
    so int8->int8 halves HBM/DMA time vs the old inline int8->bf16 cast).
    Per (slot, 2-kvh-group) the host packs one contiguous DRAM block whose
    row p is [K(d=p) | V(t=p)], so each chunk is ONE big DMA.
  - On-chip the int8 is cast to bf16 for the matmuls, rate-matched across
    the two usable elementwise engines: DVE (~1.9 fe/ns, 2x_2p mode) takes
    K plus the tail ~25% of V, ACT (~1.1 fe/ns) the rest of V.  GpSimd is
    deliberately unused: it shares an SBUF port with the DVE and concurrent
    big copies collapse both engines ~3x (measured).  Casts are issued one
    chunk ahead of compute (in-order engine queues execute in dependency-
    ready order); chunk DMAs run 6 ahead on the sync HWDGE queue.
  - Per (slot, group of 2 kv heads):
      scores [128t, 2kvh, n, 4h] = per-tile matmuls(lhsT=K^T tile, rhs=q^T)
      s1 = scores * ksb  (DVE; ksb = k_scale*SCALE, zeroed beyond ctx)
      e  = exp(s1) in bf16 (ACT), ev = e * v_scale_vec (DVE)
      Z  = matmul(lhsT=e, rhs=ones) per kvh; pad tokens contribute exp(0)=1,
           corrected host-side via the known count
      PV = matmul(lhsT=V tile, rhs=ev) accumulated in PSUM as out^T [128d,4h]
  Softmax skips max-subtraction (scores are O(20) at most; fp32 exp is safe).
"""

import math
import os
from contextlib import ExitStack

import numpy as np

import sys
sys.path.insert(0, "/opt/trn_rl_repo")

import ml_dtypes  # noqa: E402

import concourse.bass as bass  # noqa: E402
import concourse.mybir as mybir  # noqa: E402
import concourse.tile as tile  # noqa: E402
from concourse import bacc  # noqa: E402
from concourse.bass_utils import run_bass_kernel_spmd  # noqa: E402

BF16 = ml_dtypes.bfloat16

B = 32
NUM_HEADS = 32
KVH = 8
D = 128
REP = NUM_HEADS // KVH  # 4
BLOCK_SIZE = 256
T = 4096
P = 128
SCALE = 1.0 / float(np.sqrt(D))
NCORES = 8

# per-chunk int8 bytes for a (2-kvh, n-tile) group: [K | V] rows
def _chunk_bytes(n):
    return 4 * n * P * P  # (2 kvh) * (K+V) * n tiles * 128 tok * 128 d


# ---------------------------------------------------------------------------
# host-side planning + packing
# ---------------------------------------------------------------------------

def _greedy_chunks(tiles, L):
    """Slot-by-slot, give the 8 largest remaining sequences a chunk of up to
    L[s] tiles.  Returns per-slot lists of (seq, start_tile, len) or None if
    some sequence is left uncovered."""
    rem = [int(t) for t in tiles]
    start = [0] * len(tiles)
    chunks = []
    for Ls in L:
        order = sorted(range(len(tiles)), key=lambda b: -rem[b])
        sc = []
        for c in range(NCORES):
            b = order[c]
            ln = min(rem[b], Ls)
            sc.append((b, start[b], ln))
            rem[b] -= ln
            start[b] += ln
        chunks.append(sc)
    if any(r > 0 for r in rem):
        return None
    return chunks


_PLAN_CACHE = {}


def _plan(context_lens):
    """Choose slot lengths L and the (core, slot) -> sequence-chunk map."""
    tiles = tuple(int(math.ceil(int(c) / P)) for c in context_lens)
    if tiles in _PLAN_CACHE:
        return _PLAN_CACHE[tiles]
    ts = sorted(tiles, reverse=True)
    # octile fallback (always feasible): whole sequences, 4 slots
    best = (ts[0] + ts[8] + ts[16] + ts[24], (ts[0], ts[8], ts[16], ts[24]))
    for L0 in range(max(2, ts[0] - 8), ts[0] + 1):
        for L1 in range(max(2, ts[8] - 8), min(L0, ts[8] + 4) + 1):
            for L2 in range(max(2, ts[16] - 7), min(L1, ts[16] + 4) + 1):
                for L3 in range(max(2, ts[24] - 5), min(L2, ts[24] + 4) + 1):
                    base = L0 + L1 + L2 + L3
                    for L4 in range(2, min(L3, 10) + 1):
                        for L5 in (0, *range(2, L4 + 1)):
                            l6r = (0,) if L5 == 0 else (0, *range(2, L5 + 1))
                            for L6 in l6r:
                                L = tuple(x for x in
                                          (L0, L1, L2, L3, L4, L5, L6) if x)
                                N = sum(L)
                                if N >= best[0]:
                                    continue
                                if _greedy_chunks(tiles, L) is not None:
                                    best = (N, L)
    L = list(best[1])
    chunks = _greedy_chunks(tiles, L)
    _PLAN_CACHE[tiles] = (L, chunks)
    return L, chunks


def _quantize(x):
    absmax = np.abs(x).max(axis=-1)
    scale = np.where(absmax > 0.0, absmax / 127.0, 1.0).astype(np.float32)
    xq = np.clip(np.round(x / scale[..., None]), -127.0, 127.0).astype(np.int32)
    return xq, scale


def _pack_inputs(inputs, L, chunks):
    q = inputs["q"].reshape(B, NUM_HEADS, D).astype(np.float32)
    k = inputs["k"].reshape(B, KVH, D).astype(np.float32)
    v = inputs["v"].reshape(B, KVH, D).astype(np.float32)
    kc = np.ascontiguousarray(inputs["k_cache_q"].reshape(-1, KVH, D))
    vc = np.ascontiguousarray(inputs["v_cache_q"].reshape(-1, KVH, D))
    ks = np.ascontiguousarray(inputs["k_scale"].reshape(-1, KVH)).astype(np.float32)
    vs = np.ascontiguousarray(inputs["v_scale"].reshape(-1, KVH)).astype(np.float32)
    bt = inputs["block_tables"]
    ctx = inputs["context_lens"]
    sm = inputs["slot_mapping"]

    # store_kvcache_int8: quantize the new token and scatter into the cache
    kq, ksn = _quantize(k)
    vq, vsn = _quantize(v)
    kc = kc.copy(); vc = vc.copy(); ks = ks.copy(); vs = vs.copy()
    kc[sm] = kq; vc[sm] = vq; ks[sm] = ksn; vs[sm] = vsn

    SLOTS = len(L)
    NTT = sum(L)
    offs = np.concatenate([[0], np.cumsum(L)])

    in_maps = []
    padcnt = np.zeros((NCORES, SLOTS), dtype=np.float64)
    for c in range(NCORES):
        kv_c = np.zeros((1, 16 * NTT * P * P), dtype=np.int8)
        # scales in global token-tile-major layout [P, KVH, NTT]
        ksb_c = np.zeros((P, KVH, NTT), dtype=np.float32)
        vsb_c = np.zeros((P, KVH, NTT), dtype=BF16)
        qt_c = np.zeros((P, SLOTS * 32), dtype=BF16)
        for s in range(SLOTS):
            b, t0, ln = chunks[s][c]
            n = L[s]
            nt = n * P
            o = int(offs[s])
            nvalid = max(0, min(int(ctx[b]) - t0 * P, ln * P))
            padcnt[c, s] = nt - nvalid
            if ln > 0:
                flat = (bt[b][:, None] * BLOCK_SIZE
                        + np.arange(BLOCK_SIZE, dtype=np.int64)[None, :]
                        ).reshape(-1)[t0 * P: t0 * P + ln * P]
                kg = np.zeros((nt, KVH, D), dtype=np.int8)
                vg = np.zeros((nt, KVH, D), dtype=np.int8)
                kg[: ln * P] = kc[flat]
                vg[: ln * P] = vc[flat]
                scg = np.zeros((nt, KVH), dtype=np.float32)
                svg = np.zeros((nt, KVH), dtype=np.float32)
                valid = (np.arange(nt) < nvalid)
                scg[: ln * P] = ks[flat] * SCALE
                svg[: ln * P] = vs[flat]
                scg *= valid[:, None]
                svg *= valid[:, None]
                kjdt = kg.transpose(1, 2, 0)                      # [KVH, D, nt]
                vpjid = vg.reshape(n, P, KVH, D).transpose(1, 2, 0, 3)
                for jh in range(KVH // 2):
                    co = 16 * o * P * P + _chunk_bytes(n) * jh
                    kb = kjdt[2 * jh: 2 * jh + 2].transpose(1, 0, 2
                                                            ).reshape(P, -1)
                    vb = vpjid[:, 2 * jh: 2 * jh + 2].reshape(P, -1)
                    kv_c[0, co: co + 4 * n * P * P] = np.concatenate(
                        [kb, vb], axis=1).reshape(-1)

                def sprd(a, dt):
                    return a.reshape(n, P, KVH).transpose(1, 2, 0).astype(dt)
                ksb_c[:, :, o: o + n] = sprd(scg, np.float32)
                vsb_c[:, :, o: o + n] = sprd(svg, BF16)
            qt_c[:, s * 32: (s + 1) * 32] = q[b].transpose(1, 0)  # [D, 32]
        in_maps.append(dict(kv=kv_c, ksb=ksb_c.reshape(P, -1),
                            vsb=vsb_c.reshape(P, -1), qt=qt_c))
    return in_maps, padcnt


# ---------------------------------------------------------------------------
# device program
# ---------------------------------------------------------------------------

# cast-engine throughput estimates (free-elems per ns) + per-op fixed ns,
# used only for the static greedy load balancer.  GpSimd is EXCLUDED: it
# shares an SBUF port with the vector engine, and concurrent big gpsimd
# copies collapse both engines to ~1/3 throughput (measured).
CAST_RATE = {"v": 1.92, "s": 1.20}
CAST_FIX = {"v": 170.0, "s": 400.0}


def _build_program(L):
    SLOTS = len(L)
    NTT = sum(L)
    offs = [0]
    for n in L:
        offs.append(offs[-1] + n)
    f32 = mybir.dt.float32
    bf16 = mybir.dt.bfloat16
    i8 = mybir.dt.int8
    EXP = mybir.ActivationFunctionType.Exp

    nc = bacc.Bacc("TRN2", target_bir_lowering=False, debug=False,
                   num_devices=NCORES)

    kv_d = nc.dram_tensor("kv", [1, 16 * NTT * P * P], i8,
                          kind="ExternalInput").ap()
    ksb_d = nc.dram_tensor("ksb", [P, KVH * NTT], f32, kind="ExternalInput").ap()
    vsb_d = nc.dram_tensor("vsb", [P, KVH * NTT], bf16, kind="ExternalInput").ap()
    qt_d = nc.dram_tensor("qt", [P, SLOTS * 32], bf16, kind="ExternalInput").ap()
    po_d = nc.dram_tensor("po", [P, SLOTS * 40], f32, kind="ExternalOutput").ap()

    with tile.TileContext(nc) as tc, ExitStack() as ctx:
        const = ctx.enter_context(tc.tile_pool(name="const", bufs=1))
        p8 = ctx.enter_context(tc.tile_pool(name="p8", bufs=7))
        pb = ctx.enter_context(tc.tile_pool(name="pb", bufs=4))
        work = ctx.enter_context(tc.tile_pool(name="wrk", bufs=5))
        ps_qk = ctx.enter_context(tc.tile_pool(name="psqk", bufs=3, space="PSUM"))
        ps_pt = ctx.enter_context(tc.tile_pool(name="pspt", bufs=2, space="PSUM"))
        ps_pv = ctx.enter_context(tc.tile_pool(name="pspv", bufs=3, space="PSUM"))

        qt = const.tile([P, SLOTS * 32], bf16)
        nc.scalar.dma_start(qt, qt_d)
        ones = const.tile([P, 1], bf16)
        nc.gpsimd.memset(ones, 1.0)
        # prewarm the ACT exp table set so the ~1.3us load is not paid
        # mid-pipeline on the first real exp
        dm = const.tile([P, 1], bf16)
        nc.scalar.activation(dm, ones, EXP)
        # scale vectors for ALL slots in two DMAs on the scalar queue
        # (layout [P, KVH, NTT]: global token-tile index inner)
        ksb_a = const.tile([P, KVH, NTT, 1], f32)
        nc.scalar.dma_start(ksb_a, ksb_d)
        vsb_a = const.tile([P, KVH, NTT, 1], bf16)
        nc.scalar.dma_start(vsb_a, vsb_d)
        # all per-slot outputs accumulate here; ONE out-DMA at the end
        po = const.tile([P, SLOTS * 40], f32)
        nc.gpsimd.memset(po, 0.0)

        # slot order: two medium slots first (enough cast work to cover the
        # big slots' DMA ramp), big slots in the middle, smallest slots last
        # (short serial tail)
        if SLOTS >= 6:
            slot_order = [3, 4] + [0, 1, 2] + list(range(5, SLOTS))
        else:
            slot_order = list(range(SLOTS))
        chunk_list = [(s, jh) for s in slot_order for jh in range(KVH // 2)]

        t8s = {}

        def issue_dma(ci):
            """DMA one (slot, 2-kvh) chunk as raw int8 (sync HWDGE queue)."""
            s, jh = chunk_list[ci]
            n = L[s]
            o = offs[s]
            co = 16 * o * P * P + _chunk_bytes(n) * jh
            t8 = p8.tile([P, 2, 2, n, P], i8, tag="kv8", name="t8")
            nc.sync.dma_start(
                t8, kv_d[0:1, co: co + 4 * n * P * P].rearrange(
                    "o (d r) -> (o d) r", d=P))
            t8s[ci] = t8

        def issue_casts(ci):
            """int8->bf16, rate-matched: DVE takes K + the tail ~25% of V,
            ACT the rest of V -> both engines finish each chunk's cast in
            about the same wall time."""
            s, jh = chunk_list[ci]
            n = L[s]
            t8 = t8s.pop(ci)
            tb = pb.tile([P, 2, 2, n, P], bf16, tag="kvb", name="tb")
            dn = int(round(0.22 * n)) if n >= 8 else 0
            dm = 2 if n >= 13 else 0  # tiles cast by the DMA fabric itself
            a = n - dn - dm
            nc.vector.tensor_copy(tb[:, 0], t8[:, 0])            # K on DVE
            if dn > 0:
                nc.vector.tensor_copy(tb[:, 1, :, n - dn:, :],
                                      t8[:, 1, :, n - dn:, :])   # V tail DVE
            if dm > 0:
                # SWDGE SBUF->SBUF casting DMA: moves V-cast work onto the
                # DMA engines' slack (idle Q7 generates the descriptors)
                nc.gpsimd.dma_start(tb[:, 1, :, a: n - dn, :],
                                    t8[:, 1, :, a: n - dn, :])
            nc.scalar.copy(tb[:, 1, :, : a, :],
                           t8[:, 1, :, : a, :])                  # V head ACT
            return tb

        slot_state = {}

        def issue_compute(cis, tbs):
            """Compute for a PAIR of chunks (4 kv heads) at once: one
            s1-mul / exp / ev-mul over [P, 4, n, 4] halves the elementwise
            op count (fixed per-op costs dominate these small ops)."""
            s, jh0 = chunk_list[cis[0]]
            n = L[s]
            o = offs[s]
            if jh0 == 0:
                slot_state[s] = (
                    ps_pv.tile([P, 32], f32, tag="pv", name="pv"),
                    ps_pt.tile([4 * n, KVH], f32, tag="pt", name="pt"),
                )
            pv, pt = slot_state[s]
            G = 2 * len(cis)  # kv heads in this group

            qk = ps_qk.tile([P, G, n, 4], f32, tag="qk")
            for ii, ci in enumerate(cis):
                tb = tbs[ii]
                for j2 in range(2):
                    j = 2 * jh0 + 2 * ii + j2
                    qcol = s * 32 + 4 * j
                    for i in range(n):
                        nc.tensor.matmul(
                            qk[:, 2 * ii + j2, i, :],
                            lhsT=tb[:, 0, j2, i, :],
                            rhs=qt[:, qcol: qcol + 4],
                            start=True, stop=True, skip_group_check=True)

            s1 = work.tile([P, G, n, 4], f32, tag="s1")
            nc.vector.tensor_mul(
                s1, qk,
                ksb_a[:, 2 * jh0: 2 * jh0 + G, o: o + n].to_broadcast(
                    [P, G, n, 4]))
            e = work.tile([P, G, n, 4], bf16, tag="e")
            nc.scalar.activation(e, s1, EXP)
            ev = work.tile([P, G, n, 4], bf16, tag="ev")
            nc.vector.tensor_mul(
                ev, e,
                vsb_a[:, 2 * jh0: 2 * jh0 + G, o: o + n].to_broadcast(
                    [P, G, n, 4]))

            for ii, ci in enumerate(cis):
                tb = tbs[ii]
                for j2 in range(2):
                    j = 2 * jh0 + 2 * ii + j2
                    # Z partials: per-(tile, head) column sums of e
                    nc.tensor.matmul(
                        pt[:, j: j + 1],
                        lhsT=e[:, 2 * ii + j2], rhs=ones,
                        start=True, stop=True, skip_group_check=True)
                    # PV accumulate over token tiles: out^T [128d, 4h]
                    cc = 4 * j
                    for i in range(n):
                        nc.tensor.matmul(
                            pv[:, cc: cc + 4],
                            lhsT=tb[:, 1, j2, i, :],
                            rhs=ev[:, 2 * ii + j2, i, :],
                            start=(i == 0), stop=(i == n - 1),
                            skip_group_check=True)

        def issue_slot_end(ci):
            s, jh = chunk_list[ci]
            if jh != KVH // 2 - 1:
                return
            n = L[s]
            pv, pt = slot_state.pop(s)
            # slot done: stage PV and raw Z partials; host does the fold
            nc.vector.tensor_copy(po[:, s * 40: s * 40 + 32], pv)
            nc.scalar.copy(po[0: 4 * n, s * 40 + 32: s * 40 + 40], pt)

        # software-pipelined main loop over PAIRS of chunks.  Per-engine
        # queue order is the expected dependency-ready order (in-order
        # queues): casts for the next chunks (their DMAs ran DMA_AHEAD
        # chunks ahead), then compute for the current pair, then slot-end
        # PSUM drains.
        NCH = len(chunk_list)
        DMA_AHEAD = 7
        for j in range(min(DMA_AHEAD, NCH)):
            issue_dma(j)
        tb_cur = [issue_casts(0), issue_casts(1)]
        for p in range(NCH // 2):
            c0 = 2 * p
            for c in (c0 + DMA_AHEAD, c0 + 1 + DMA_AHEAD):
                if c < NCH:
                    issue_dma(c)
            tb_next = ([issue_casts(c0 + 2), issue_casts(c0 + 3)]
                       if c0 + 2 < NCH else None)
            issue_compute((c0, c0 + 1), tb_cur)
            # drain the PREVIOUS pair's finished slot (its PV/Z are long
            # done, so these PSUM reads never block the cast queue)
            if p > 0:
                issue_slot_end(c0 - 1)
            tb_cur = tb_next
        issue_slot_end(NCH - 1)

        nc.scalar.dma_start(po_d, po)

    nc.compile()
    return nc


_PROGRAM_CACHE = {}


def _get_program(L):
    key = tuple(L)
    if key not in _PROGRAM_CACHE:
        _PROGRAM_CACHE[key] = _build_program(L)
    return _PROGRAM_CACHE[key]


# ---------------------------------------------------------------------------
# entry point
# ---------------------------------------------------------------------------

def kernel(q, k, v, k_cache_q, v_cache_q, k_scale, v_scale,
           block_tables, context_lens, slot_mapping, _trace=False):
    inputs = dict(q=np.asarray(q), k=np.asarray(k), v=np.asarray(v),
                  k_cache_q=np.asarray(k_cache_q),
                  v_cache_q=np.asarray(v_cache_q),
                  k_scale=np.asarray(k_scale), v_scale=np.asarray(v_scale),
                  block_tables=np.asarray(block_tables),
                  context_lens=np.asarray(context_lens),
                  slot_mapping=np.asarray(slot_mapping))
    L, chunks = _plan(inputs["context_lens"])
    in_maps, padcnt = _pack_inputs(inputs, L, chunks)
    nc = _get_program(L)
    res = run_bass_kernel_spmd(nc, in_maps, core_ids=list(range(NCORES)),
                               trace=_trace)

    # combine unnormalized partials across chunks (flash-decoding merge)
    accp = np.zeros((B, P, 32), dtype=np.float64)
    accz = np.zeros((B, 32), dtype=np.float64)
    for c in range(NCORES):
        po = res.results[c]["po"]    # [P, SLOTS*40]
        for s in range(len(L)):
            b, _, _ = chunks[s][c]
            n = L[s]
            accp[b] += po[:, s * 40: s * 40 + 32]
            # raw Z partials [4n, KVH]: row r = tile i*4 + head h
            pt = po[0: 4 * n, s * 40 + 32: s * 40 + 40]
            z32 = pt.reshape(n, 4, KVH).sum(axis=0).T.reshape(32)
            accz[b] += z32 - padcnt[c, s]
    out = (accp / accz[:, None, :]).transpose(0, 2, 1)  # [B, 32h, 128d]
    out = np.ascontiguousarray(out.reshape(B, NUM_HEADS * D), dtype=np.float32)
    if _trace:
        return out, res
    return out
